# revision 4
# baseline (speedup 1.0000x reference)
"""Spatial-reduction attention (PVT-style) on 8 Trainium2 NeuronCores — v2.

Shapes (hardcoded): x [4, 4096, 512], 8 heads, head_dim 64, SR=2 conv
reduction -> 1024 keys. Sharding: core c handles batch c//2, query half
c%2 (2048 queries). Conv + kv recomputed per core pair.

v2 changes over baseline:
  - scores row-tiled: head-even at PE rows 0:63, head-odd at rows 64:127
    (tile_position (0,0)/(64,0)) run CONCURRENTLY -> 2x scores throughput.
    kT packed as [128, kt, keys] (even head dims in partitions 0:63, odd in
    64:127) with no zero padding.
  - one exp per (pr,nk) over a single [128, 2048] PSUM tile (4 banks).
  - attnV col-tiled: v-even (M=64) -> PSUM partitions 0:63, v-odd ->
    64:127 of the SAME bank, concurrent. No ones-column.
  - softmax denominators via 4 concurrent M=1 col-tiled ones-matmuls into
    a separate PSUM bank (rows 0/32/64/96), accumulated over nk.
  - normalize: copy o-banks to SBUF bf16 immediately (frees banks), DMA-
    gather denom rows -> [4,512] reciprocal on DVE -> partition_broadcast
    -> two full-128-lane multiplies in place.
  - PSUM budget: s01 4 banks + o 2 + d 1 + work 1 = 8. proj/late-qT groups
    use the work bank, interleaved into the exp shadow.
"""

import numpy as np
import ml_dtypes
from contextlib import ExitStack

import concourse.bass as bass
import concourse.mybir as mybir
from concourse import bacc
from concourse.bass_utils import run_bass_kernel_spmd
from concourse.tile import TileContext

BF = mybir.dt.bfloat16
F32 = mybir.dt.float32
P = 128
CT = 4            # channel tiles (512 / 128)
NQ = 2048         # queries per core
NKT = 8           # key tiles (1024 / 128)
SCALE = 0.125     # 64 ** -0.5

_CACHE = {}
DEBUG_DUMP = False


def _build_program():
    nc = bacc.Bacc("TRN2", target_bir_lowering=False, debug=False, num_devices=8)

    xq_d = nc.dram_tensor("xq", [512, NQ], BF, kind="ExternalInput")
    xf_d = nc.dram_tensor("xf", [512, 4096], BF, kind="ExternalInput")
    qw_d = nc.dram_tensor("qw", [512, 512], BF, kind="ExternalInput")      # [c, dq]
    kw_d = nc.dram_tensor("kw", [512, 512], BF, kind="ExternalInput")      # [c, dk]
    vw_d = nc.dram_tensor("vw", [512, 512], BF, kind="ExternalInput")      # [c, dv]
    srw_d = nc.dram_tensor("srw", [4, 512, 512], BF, kind="ExternalInput")  # [ij, ci, co]
    srb_d = nc.dram_tensor("srb", [512], F32, kind="ExternalInput")
    pw_d = nc.dram_tensor("pw", [512, 512], BF, kind="ExternalInput")      # [c, co]
    pb_d = nc.dram_tensor("pb", [512], F32, kind="ExternalInput")
    out_d = nc.dram_tensor("out_t", [512, NQ], F32, kind="ExternalOutput")
    if DEBUG_DUMP:
        dbg_qT = nc.dram_tensor("dbg_qT", [P, CT, NQ], BF, kind="ExternalOutput")
        dbg_conv = nc.dram_tensor("dbg_conv", [P, CT, 1024], BF, kind="ExternalOutput")
        dbg_kT2 = nc.dram_tensor("dbg_kT2", [P, CT, 1024], BF, kind="ExternalOutput")
        dbg_v = nc.dram_tensor("dbg_v", [P, NKT, 8, 64], BF, kind="ExternalOutput")
        dbg_oTu = nc.dram_tensor("dbg_oTu", [P, CT, NQ], BF, kind="ExternalOutput")
        dbg_oT = nc.dram_tensor("dbg_oT", [P, CT, NQ], BF, kind="ExternalOutput")
        dbg_rpk = nc.dram_tensor("dbg_rpk", [8, 4, 512], F32, kind="ExternalOutput")
        dbg_e = nc.dram_tensor("dbg_e", [P, 2048], BF, kind="ExternalOutput")

    Exp = mybir.ActivationFunctionType.Exp

    with TileContext(nc) as tc, ExitStack() as ctx:
        const = ctx.enter_context(tc.tile_pool(name="const", bufs=1))
        expp = ctx.enter_context(tc.tile_pool(name="expp", bufs=3))
        dpkp = ctx.enter_context(tc.tile_pool(name="dpkp", bufs=1))
        rbp = ctx.enter_context(tc.tile_pool(name="rbp", bufs=1))
        outp = ctx.enter_context(tc.tile_pool(name="outp", bufs=3))

        dma = nc.sync.dma_start

        # ---- ACT table prewarm (loads exp spline tables during DMA loads) ----
        warm = const.tile([1, 32], F32)
        nc.gpsimd.memset(warm, 0.0)
        warm_o = const.tile([1, 32], BF)
        nc.scalar.activation(warm_o, warm, Exp, scale=1.0)

        # ---- load inputs ----
        qw_sb = const.tile([P, CT, 512], BF)
        qw_r = qw_d.rearrange("(t p) n -> p t n", p=P)
        for t in range(CT):
            dma(out=qw_sb[:, t, :], in_=qw_r[:, t, :])
        xq_sb = const.tile([P, CT, NQ], BF)
        xq_r = xq_d.rearrange("(t p) n -> p t n", p=P)
        for t in range(CT):
            dma(out=xq_sb[:, t, :], in_=xq_r[:, t, :])
        kw_sb = const.tile([P, CT, 512], BF)
        dma(out=kw_sb, in_=kw_d.rearrange("(t p) n -> p t n", p=P))
        vw_sb = const.tile([P, CT, 512], BF)
        dma(out=vw_sb, in_=vw_d.rearrange("(t p) n -> p t n", p=P))
        srw_sb = const.tile([P, 4, CT, 512], BF)
        srw_r = srw_d.rearrange("i (t p) o -> p i t o", p=P)
        for ij4 in range(4):
            dma(out=srw_sb[:, ij4, :, :], in_=srw_r[:, ij4, :, :])
        srb_sb = const.tile([P, CT], F32)
        dma(out=srb_sb, in_=srb_d.rearrange("(t p) -> p t", p=P))
        pw_sb = const.tile([P, CT, 512], BF)
        dma(out=pw_sb, in_=pw_d.rearrange("(t p) n -> p t n", p=P))
        pb_sb = const.tile([P, CT], F32)
        dma(out=pb_sb, in_=pb_d.rearrange("(t p) -> p t", p=P))

        xf_sb = const.tile([P, CT, 4096], BF)
        xf_r = xf_d.rearrange("(t p) n -> p t n", p=P)
        for t in range(CT):
            dma(out=xf_sb[:, t, :], in_=xf_r[:, t, :])

        qT_sb = const.tile([P, CT, NQ], BF)
        convT_sb = const.tile([P, CT, 1024], BF)
        kT2_sb = const.tile([P, CT, 1024], BF)
        vaug_sb = const.tile([P, NKT, 8, 64], BF)
        oT_sb = const.tile([P, CT, NQ], BF)
        oTu_sb = const.tile([P, CT, NQ], BF)
        ones_sb = const.tile([P, 1], BF)
        nc.gpsimd.memset(ones_sb, 1.0)

        def qT_group(dq, nqb, pool, tag="work"):
            ps = pool.tile([P, 512], F32, tag=tag,
                           name=f"qt_{dq}_{nqb}")
            for c in range(CT):
                nc.tensor.matmul(
                    ps,
                    qw_sb[:, c, dq * 128:(dq + 1) * 128],
                    xq_sb[:, c, nqb * 512:(nqb + 1) * 512],
                    start=(c == 0), stop=(c == CT - 1),
                )
            nc.vector.tensor_copy(
                qT_sb[:, dq, nqb * 512:(nqb + 1) * 512], ps)

        def proj_group(co, nqb, pool, tag="work"):
            ps = pool.tile([P, 512], F32, tag=tag,
                           name=f"pj_{co}_{nqb}")
            for c in range(CT):
                nc.tensor.matmul(
                    ps,
                    pw_sb[:, c, co * 128:(co + 1) * 128],
                    oT_sb[:, c, nqb * 512:(nqb + 1) * 512],
                    start=(c == 0), stop=(c == CT - 1),
                )
            pt = outp.tile([P, 512], F32)
            nc.vector.tensor_scalar_add(pt, ps, pb_sb[:, co:co + 1])
            dma(out=out_d[co * 128:(co + 1) * 128,
                          nqb * 512:(nqb + 1) * 512], in_=pt)

        with ExitStack() as ps_ctx:
            ps1 = ps_ctx.enter_context(tc.tile_pool(name="ps1", bufs=6, space="PSUM"))

            # ---- qT for pr=0 only (rest emitted inside phase F) ----
            for nqb in range(4):
                qT_group(0, nqb, ps1, tag="ps")

            # ---- conv (spatial reduction) ----
            for co in range(CT):
                for nkb in range(2):
                    ps = ps1.tile([P, 512], F32, tag="ps",
                                  name=f"cv_{co}_{nkb}")
                    n_mm = 0
                    for ij in range(4):
                        i, j = ij >> 1, ij & 1
                        for ci in range(CT):
                            rhs = xf_sb[:, ci, :].rearrange(
                                "p (a i b j) -> p i j a b", a=32, i=2, b=32, j=2
                            )[:, i, j, nkb * 16:(nkb + 1) * 16, :]
                            nc.tensor.matmul(
                                ps,
                                srw_sb[:, ij, ci, co * 128:(co + 1) * 128],
                                rhs,
                                start=(n_mm == 0), stop=(n_mm == 15),
                            )
                            n_mm += 1
                    nc.vector.tensor_scalar_add(
                        convT_sb[:, co, nkb * 512:(nkb + 1) * 512],
                        ps, srb_sb[:, co:co + 1])

            # ---- kT = k_wT.T @ convT (pair-packed layout) ----
            for kt in range(CT):
                for nkb in range(2):
                    ps = ps1.tile([P, 512], F32, tag="ps",
                                  name=f"kt_{kt}_{nkb}")
                    for c in range(CT):
                        nc.tensor.matmul(
                            ps,
                            kw_sb[:, c, kt * 128:(kt + 1) * 128],
                            convT_sb[:, c, nkb * 512:(nkb + 1) * 512],
                            start=(c == 0), stop=(c == CT - 1),
                        )
                    nc.vector.tensor_copy(
                        kT2_sb[:, kt, nkb * 512:(nkb + 1) * 512], ps)

            # ---- v = convT.T @ v_wT ----
            for nk in range(NKT):
                ps = ps1.tile([P, 512], F32, tag="ps",
                              name=f"v_{nk}")
                for c in range(CT):
                    nc.tensor.matmul(
                        ps,
                        convT_sb[:, c, nk * 128:(nk + 1) * 128],
                        vw_sb[:, c, :],
                        start=(c == 0), stop=(c == CT - 1),
                    )
                nc.vector.tensor_copy(
                    vaug_sb[:, nk, :, :],
                    ps.rearrange("p (h e) -> p h e", e=64),
                )

        # ---- phase F: attention rounds ----
        with ExitStack() as ps_ctx:
            ps_s = ps_ctx.enter_context(
                tc.tile_pool(name="ps_s", bufs=1, space="PSUM"))
            ps_o = ps_ctx.enter_context(
                tc.tile_pool(name="ps_o", bufs=1, space="PSUM"))
            ps_d = ps_ctx.enter_context(
                tc.tile_pool(name="ps_d", bufs=1, space="PSUM"))
            ps_w = ps_ctx.enter_context(
                tc.tile_pool(name="ps_w", bufs=1, space="PSUM"))

            for hf in range(2):
                for pr in range(4):
                    # late qT: emit right before the round that needs it; runs
                    # in the exp shadow of the previous round.
                    if hf == 0 and pr >= 1:
                        for nqb in range(4):
                            qT_group(pr, nqb, ps_w)
                    # interleave proj(hf=0) groups into hf=1 rounds pr=0,1
                    # proj for hf=0 queries (nqb 0,1) interleaved into the
                    # first two hf=1 rounds; they read oT columns written by
                    # the (complete) hf=0 rounds.
                    fillers = []
                    if hf == 1 and pr < 2:
                        fillers = [(co, pr) for co in range(CT)]

                    o_ps = [ps_o.tile([P, 512], F32, tag=f"o{q2}",
                                      name=f"o_{hf}_{pr}_{q2}")
                            for q2 in range(2)]
                    d_ps = ps_d.tile([P, 512], F32, tag="d",
                                     name=f"d_{hf}_{pr}")
                    for nk in range(NKT):
                        s01 = ps_s.tile([P, 2048], F32, tag="s01",
                                        name=f"s_{hf}_{pr}_{nk}")
                        nks = slice(nk * 128, (nk + 1) * 128)
                        for q2 in range(2):
                            nqs = hf * 1024 + q2 * 512
                            # head-even: PE rows 0:63; head-odd: rows 64:127
                            nc.tensor.matmul(
                                s01[:, q2 * 512:(q2 + 1) * 512],
                                kT2_sb[0:64, pr, nks],
                                qT_sb[0:64, pr, nqs:nqs + 512],
                                start=True, stop=True,
                            )
                            nc.tensor.matmul(
                                s01[:, 1024 + q2 * 512:1024 + (q2 + 1) * 512],
                                kT2_sb[64:128, pr, nks],
                                qT_sb[64:128, pr, nqs:nqs + 512],
                                start=True, stop=True,
                            )
                        e01 = expp.tile([P, 2048], BF)
                        nc.scalar.activation(e01, s01, Exp, scale=SCALE)
                        if DEBUG_DUMP and hf == 0 and pr == 0 and nk == 0:
                            dma(out=dbg_e[:, :], in_=e01)
                        for q2 in range(2):
                            # attnV col-tiled pair: even head -> partitions
                            # 0:63, odd head -> 64:127 of the same bank
                            nc.tensor.matmul(
                                o_ps[q2][0:64, :],
                                vaug_sb[:, nk, 2 * pr, :],
                                e01[:, q2 * 512:(q2 + 1) * 512],
                                start=(nk == 0), stop=(nk == NKT - 1),
                                skip_group_check=True,
                            )
                            nc.tensor.matmul(
                                o_ps[q2][64:128, :],
                                vaug_sb[:, nk, 2 * pr + 1, :],
                                e01[:, 1024 + q2 * 512:1024 + (q2 + 1) * 512],
                                start=(nk == 0), stop=(nk == NKT - 1),
                                skip_group_check=True,
                            )
                        # denominators: 4 concurrent M=1 col-tiled matmuls
                        for q2 in range(2):
                            for h2 in range(2):
                                r = 32 * (2 * q2 + h2)
                                nc.tensor.matmul(
                                    d_ps[r:r + 1, :],
                                    ones_sb,
                                    e01[:, h2 * 1024 + q2 * 512:
                                        h2 * 1024 + (q2 + 1) * 512],
                                    start=(nk == 0), stop=(nk == NKT - 1),
                                    tile_position=(0, r),
                                    skip_group_check=True,
                                )
                        if fillers:
                            proj_group(*fillers.pop(), ps_w)

                    # ---- round end: free o banks fast, then normalize ----
                    for q2 in range(2):
                        hq = hf * 1024 + q2 * 512
                        nc.vector.tensor_copy(
                            oTu_sb[:, pr, hq:hq + 512], o_ps[q2])
                    # reciprocal_approx_fast cannot read PSUM on HW: copy
                    # each denom row to SBUF first. partition_broadcast into a
                    # base-64 half is broken on HW: broadcast odd-head recip
                    # to a full 128-partition tile and slice.
                    rpk = [dpkp.tile([1, 512], F32, tag=f"rpk{i}",
                                     name=f"rpk_{hf}_{pr}_{i}")
                           for i in range(4)]
                    for i in range(4):
                        dcp = dpkp.tile([1, 512], F32, tag="dcp",
                                        name=f"dcp_{hf}_{pr}_{i}")
                        nc.vector.tensor_copy(
                            dcp, d_ps[32 * i:32 * i + 1, :])
                        nc.vector.reciprocal_approx_fast(
                            out=rpk[i], in_=dcp)
                    if DEBUG_DUMP:
                        for i in range(4):
                            dma(out=dbg_rpk[4 * hf + pr, i, :], in_=rpk[i])
                    for q2 in range(2):
                        rbe = rbp.tile([64, 512], F32, tag=f"rbe{q2}",
                                       name=f"rbe_{hf}_{pr}_{q2}")
                        rbo = rbp.tile([P, 512], F32, tag=f"rbo{q2}",
                                       name=f"rbo_{hf}_{pr}_{q2}")
                        nc.gpsimd.partition_broadcast(rbe, rpk[2 * q2])
                        nc.gpsimd.partition_broadcast(rbo, rpk[2 * q2 + 1])
                        hq = hf * 1024 + q2 * 512
                        nc.vector.tensor_mul(
                            oT_sb[0:64, pr, hq:hq + 512],
                            oTu_sb[0:64, pr, hq:hq + 512], rbe)
                        nc.vector.tensor_mul(
                            oT_sb[64:128, pr, hq:hq + 512],
                            oTu_sb[64:128, pr, hq:hq + 512],
                            rbo[64:128, :])
                    while fillers:
                        proj_group(*fillers.pop(), ps_w)

        if DEBUG_DUMP:
            dma(out=dbg_qT[:, :, :], in_=qT_sb)
            dma(out=dbg_conv[:, :, :], in_=convT_sb)
            dma(out=dbg_kT2[:, :, :], in_=kT2_sb)
            dma(out=dbg_v[:, :, :, :], in_=vaug_sb)
            dma(out=dbg_oTu[:, :, :], in_=oTu_sb)
            dma(out=dbg_oT[:, :, :], in_=oT_sb)

        # ---- tail: proj for hf=1 queries with freed PSUM banks ----
        with ExitStack() as ps_ctx:
            ps_t = ps_ctx.enter_context(
                tc.tile_pool(name="ps_t", bufs=3, space="PSUM"))
            for nqb in (2, 3):
                for co in range(CT):
                    proj_group(co, nqb, ps_t)

    nc.compile()
    return nc


def kernel(x, q_w, kv_w, sr_w, sr_b, proj_w, proj_b, H=64, W=64, **_kw):
    x = np.asarray(x, dtype=np.float32)
    q_w = np.asarray(q_w, dtype=np.float32)
    kv_w = np.asarray(kv_w, dtype=np.float32)
    sr_w = np.asarray(sr_w, dtype=np.float32)
    sr_b = np.asarray(sr_b, dtype=np.float32)
    proj_w = np.asarray(proj_w, dtype=np.float32)
    proj_b = np.asarray(proj_b, dtype=np.float32)
    B, N, C = x.shape

    if "nc" not in _CACHE:
        _CACHE["nc"] = _build_program()
    nc = _CACHE["nc"]

    bf = ml_dtypes.bfloat16
    qw_t = np.ascontiguousarray(q_w.T).astype(bf)              # [c, dq]
    kw_t = np.ascontiguousarray(kv_w[:512].T).astype(bf)       # [c, dk]
    vw_t = np.ascontiguousarray(kv_w[512:].T).astype(bf)       # [c, dv]
    srw_t = np.ascontiguousarray(
        sr_w.transpose(2, 3, 1, 0).reshape(4, 512, 512)).astype(bf)
    pw_t = np.ascontiguousarray(proj_w.T).astype(bf)           # [c, co]

    in_maps = []
    xT = np.ascontiguousarray(x.transpose(0, 2, 1)).astype(bf)  # [B, C, N]
    for c in range(8):
        b, hf = c // 2, c % 2
        in_maps.append({
            "xq": np.ascontiguousarray(xT[b][:, hf * NQ:(hf + 1) * NQ]),
            "xf": xT[b],
            "qw": qw_t, "kw": kw_t, "vw": vw_t,
            "srw": srw_t, "srb": sr_b,
            "pw": pw_t, "pb": proj_b,
        })

    res = run_bass_kernel_spmd(nc, in_maps, core_ids=list(range(8)))
    _CACHE["last_exec_time_ns"] = res.exec_time_ns

    out = np.empty((B, N, C), dtype=np.float32)
    for c in range(8):
        b, hf = c // 2, c % 2
        out[b, hf * NQ:(hf + 1) * NQ, :] = res.results[c]["out_t"].T
    return out


# revision 5
# speedup vs baseline: 1.3290x; 1.3290x over previous
"""Spatial-reduction attention (PVT-style) on 8 Trainium2 NeuronCores — v2.

Shapes (hardcoded): x [4, 4096, 512], 8 heads, head_dim 64, SR=2 conv
reduction -> 1024 keys. Sharding: core c handles batch c//2, query half
c%2 (2048 queries). Conv + kv recomputed per core pair.

v2 changes over baseline:
  - scores row-tiled: head-even at PE rows 0:63, head-odd at rows 64:127
    (tile_position (0,0)/(64,0)) run CONCURRENTLY -> 2x scores throughput.
    kT packed as [128, kt, keys] (even head dims in partitions 0:63, odd in
    64:127) with no zero padding.
  - one exp per (pr,nk) over a single [128, 2048] PSUM tile (4 banks).
  - attnV col-tiled: v-even (M=64) -> PSUM partitions 0:63, v-odd ->
    64:127 of the SAME bank, concurrent. No ones-column.
  - softmax denominators via 4 concurrent M=1 col-tiled ones-matmuls into
    a separate PSUM bank (rows 0/32/64/96), accumulated over nk.
  - normalize: copy o-banks to SBUF bf16 immediately (frees banks), DMA-
    gather denom rows -> [4,512] reciprocal on DVE -> partition_broadcast
    -> two full-128-lane multiplies in place.
  - PSUM budget: s01 4 banks + o 2 + d 1 + work 1 = 8. proj/late-qT groups
    use the work bank, interleaved into the exp shadow.
"""

import numpy as np
import ml_dtypes
from contextlib import ExitStack

import concourse.bass as bass
import concourse.mybir as mybir
from concourse import bacc
from concourse.bass_utils import run_bass_kernel_spmd
from concourse.tile import TileContext

BF = mybir.dt.bfloat16
F32 = mybir.dt.float32
P = 128
CT = 4            # channel tiles (512 / 128)
NQ = 2048         # queries per core
NKT = 8           # key tiles (1024 / 128)
SCALE = 0.125     # 64 ** -0.5

_CACHE = {}
DEBUG_DUMP = False


def _build_program():
    nc = bacc.Bacc("TRN2", target_bir_lowering=False, debug=False, num_devices=8)

    xq_d = nc.dram_tensor("xq", [512, NQ], BF, kind="ExternalInput")
    xf_d = nc.dram_tensor("xf", [512, 4096], BF, kind="ExternalInput")
    qw_d = nc.dram_tensor("qw", [512, 512], BF, kind="ExternalInput")      # [c, dq]
    kw_d = nc.dram_tensor("kw", [512, 512], BF, kind="ExternalInput")      # [c, dk]
    vw_d = nc.dram_tensor("vw", [512, 512], BF, kind="ExternalInput")      # [c, dv]
    srw_d = nc.dram_tensor("srw", [4, 512, 512], BF, kind="ExternalInput")  # [ij, ci, co]
    srb_d = nc.dram_tensor("srb", [512], F32, kind="ExternalInput")
    pw_d = nc.dram_tensor("pw", [512, 512], BF, kind="ExternalInput")      # [c, co]
    pb_d = nc.dram_tensor("pb", [512], F32, kind="ExternalInput")
    out_d = nc.dram_tensor("out_t", [512, NQ], F32, kind="ExternalOutput")
    if DEBUG_DUMP:
        dbg_qT = nc.dram_tensor("dbg_qT", [P, CT, NQ], BF, kind="ExternalOutput")
        dbg_conv = nc.dram_tensor("dbg_conv", [P, CT, 1024], BF, kind="ExternalOutput")
        dbg_kT2 = nc.dram_tensor("dbg_kT2", [P, CT, 1024], BF, kind="ExternalOutput")
        dbg_v = nc.dram_tensor("dbg_v", [P, NKT, 8, 64], BF, kind="ExternalOutput")
        dbg_oTu = nc.dram_tensor("dbg_oTu", [P, CT, NQ], BF, kind="ExternalOutput")
        dbg_oT = nc.dram_tensor("dbg_oT", [P, CT, NQ], BF, kind="ExternalOutput")
        dbg_rpk = nc.dram_tensor("dbg_rpk", [8, 4, 512], F32, kind="ExternalOutput")
        dbg_e = nc.dram_tensor("dbg_e", [P, 2048], BF, kind="ExternalOutput")

    Exp = mybir.ActivationFunctionType.Exp

    with TileContext(nc) as tc, ExitStack() as ctx:
        const = ctx.enter_context(tc.tile_pool(name="const", bufs=1))
        expp = ctx.enter_context(tc.tile_pool(name="expp", bufs=3))
        dpkp = ctx.enter_context(tc.tile_pool(name="dpkp", bufs=1))
        rbp = ctx.enter_context(tc.tile_pool(name="rbp", bufs=1))
        outp = ctx.enter_context(tc.tile_pool(name="outp", bufs=3))

        dma = nc.sync.dma_start

        # ---- ACT table prewarm (loads exp spline tables during DMA loads) ----
        warm = const.tile([1, 32], F32)
        nc.gpsimd.memset(warm, 0.0)
        warm_o = const.tile([1, 32], BF)
        nc.scalar.activation(warm_o, warm, Exp, scale=1.0)

        # ---- load inputs ----
        qw_sb = const.tile([P, CT, 512], BF)
        qw_r = qw_d.rearrange("(t p) n -> p t n", p=P)
        for t in range(CT):
            dma(out=qw_sb[:, t, :], in_=qw_r[:, t, :])
        xq_sb = const.tile([P, CT, NQ], BF)
        xq_r = xq_d.rearrange("(t p) n -> p t n", p=P)
        for t in range(CT):
            dma(out=xq_sb[:, t, :], in_=xq_r[:, t, :])
        kw_sb = const.tile([P, CT, 512], BF)
        dma(out=kw_sb, in_=kw_d.rearrange("(t p) n -> p t n", p=P))
        vw_sb = const.tile([P, CT, 512], BF)
        dma(out=vw_sb, in_=vw_d.rearrange("(t p) n -> p t n", p=P))
        srw_sb = const.tile([P, 4, CT, 512], BF)
        srw_r = srw_d.rearrange("i (t p) o -> p i t o", p=P)
        for ij4 in range(4):
            dma(out=srw_sb[:, ij4, :, :], in_=srw_r[:, ij4, :, :])
        srb_sb = const.tile([P, CT], F32)
        dma(out=srb_sb, in_=srb_d.rearrange("(t p) -> p t", p=P))
        pw_sb = const.tile([P, CT, 512], BF)
        dma(out=pw_sb, in_=pw_d.rearrange("(t p) n -> p t n", p=P))
        pb_sb = const.tile([P, CT], F32)
        dma(out=pb_sb, in_=pb_d.rearrange("(t p) -> p t", p=P))

        xf_sb = const.tile([P, CT, 4096], BF)
        xf_r = xf_d.rearrange("(t p) n -> p t n", p=P)
        for t in range(CT):
            dma(out=xf_sb[:, t, :], in_=xf_r[:, t, :])

        qT_sb = const.tile([P, CT, NQ], BF)
        convT_sb = const.tile([P, CT, 1024], BF)
        kT2_sb = const.tile([P, CT, 1024], BF)
        vaug_sb = const.tile([P, NKT, 8, 64], BF)
        oT_sb = const.tile([P, CT, NQ], BF)
        oTu_sb = const.tile([P, CT, NQ], BF)
        ones_sb = const.tile([P, 1], BF)
        nc.gpsimd.memset(ones_sb, 1.0)

        def qT_group(dq, nqb, pool, tag="work"):
            ps = pool.tile([P, 512], F32, tag=tag,
                           name=f"qt_{dq}_{nqb}")
            for c in range(CT):
                nc.tensor.matmul(
                    ps,
                    qw_sb[:, c, dq * 128:(dq + 1) * 128],
                    xq_sb[:, c, nqb * 512:(nqb + 1) * 512],
                    start=(c == 0), stop=(c == CT - 1),
                )
            nc.vector.tensor_copy(
                qT_sb[:, dq, nqb * 512:(nqb + 1) * 512], ps)

        def proj_group(co, nqb, pool, tag="work"):
            ps = pool.tile([P, 512], F32, tag=tag,
                           name=f"pj_{co}_{nqb}")
            for c in range(CT):
                nc.tensor.matmul(
                    ps,
                    pw_sb[:, c, co * 128:(co + 1) * 128],
                    oT_sb[:, c, nqb * 512:(nqb + 1) * 512],
                    start=(c == 0), stop=(c == CT - 1),
                )
            pt = outp.tile([P, 512], F32)
            nc.vector.tensor_scalar_add(pt, ps, pb_sb[:, co:co + 1])
            dma(out=out_d[co * 128:(co + 1) * 128,
                          nqb * 512:(nqb + 1) * 512], in_=pt)

        with ExitStack() as ps_ctx:
            ps1 = ps_ctx.enter_context(tc.tile_pool(name="ps1", bufs=6, space="PSUM"))

            # ---- qT for pr=0 only (rest emitted inside phase F) ----
            for nqb in range(4):
                qT_group(0, nqb, ps1, tag="ps")

            # ---- conv (spatial reduction) ----
            for co in range(CT):
                for nkb in range(2):
                    ps = ps1.tile([P, 512], F32, tag="ps",
                                  name=f"cv_{co}_{nkb}")
                    n_mm = 0
                    for ij in range(4):
                        i, j = ij >> 1, ij & 1
                        for ci in range(CT):
                            rhs = xf_sb[:, ci, :].rearrange(
                                "p (a i b j) -> p i j a b", a=32, i=2, b=32, j=2
                            )[:, i, j, nkb * 16:(nkb + 1) * 16, :]
                            nc.tensor.matmul(
                                ps,
                                srw_sb[:, ij, ci, co * 128:(co + 1) * 128],
                                rhs,
                                start=(n_mm == 0), stop=(n_mm == 15),
                            )
                            n_mm += 1
                    nc.vector.tensor_scalar_add(
                        convT_sb[:, co, nkb * 512:(nkb + 1) * 512],
                        ps, srb_sb[:, co:co + 1])

            # ---- kT = k_wT.T @ convT (pair-packed layout) ----
            for kt in range(CT):
                for nkb in range(2):
                    ps = ps1.tile([P, 512], F32, tag="ps",
                                  name=f"kt_{kt}_{nkb}")
                    for c in range(CT):
                        nc.tensor.matmul(
                            ps,
                            kw_sb[:, c, kt * 128:(kt + 1) * 128],
                            convT_sb[:, c, nkb * 512:(nkb + 1) * 512],
                            start=(c == 0), stop=(c == CT - 1),
                        )
                    nc.vector.tensor_copy(
                        kT2_sb[:, kt, nkb * 512:(nkb + 1) * 512], ps)

            # ---- v = convT.T @ v_wT ----
            for nk in range(NKT):
                ps = ps1.tile([P, 512], F32, tag="ps",
                              name=f"v_{nk}")
                for c in range(CT):
                    nc.tensor.matmul(
                        ps,
                        convT_sb[:, c, nk * 128:(nk + 1) * 128],
                        vw_sb[:, c, :],
                        start=(c == 0), stop=(c == CT - 1),
                    )
                nc.vector.tensor_copy(
                    vaug_sb[:, nk, :, :],
                    ps.rearrange("p (h e) -> p h e", e=64),
                )

        # ---- phase F: attention rounds ----
        with ExitStack() as ps_ctx:
            ps_s = ps_ctx.enter_context(
                tc.tile_pool(name="ps_s", bufs=1, space="PSUM"))
            ps_o = ps_ctx.enter_context(
                tc.tile_pool(name="ps_o", bufs=1, space="PSUM"))
            ps_d = ps_ctx.enter_context(
                tc.tile_pool(name="ps_d", bufs=1, space="PSUM"))
            ps_w = ps_ctx.enter_context(
                tc.tile_pool(name="ps_w", bufs=1, space="PSUM"))

            for hf in range(2):
                for pr in range(4):
                    # late qT: emit right before the round that needs it; runs
                    # in the exp shadow of the previous round.
                    if hf == 0 and pr >= 1:
                        for nqb in range(4):
                            qT_group(pr, nqb, ps_w)
                    # interleave proj(hf=0) groups into hf=1 rounds pr=0,1
                    # proj for hf=0 queries (nqb 0,1) interleaved into the
                    # first two hf=1 rounds; they read oT columns written by
                    # the (complete) hf=0 rounds.
                    fillers = []
                    if hf == 1 and pr < 2:
                        fillers = [(co, pr) for co in range(CT)]

                    o_ps = [ps_o.tile([P, 512], F32, tag=f"o{q2}",
                                      name=f"o_{hf}_{pr}_{q2}")
                            for q2 in range(2)]
                    d_ps = ps_d.tile([P, 512], F32, tag="d",
                                     name=f"d_{hf}_{pr}")
                    for nk in range(NKT):
                        # two 2-bank s tiles (even head / odd head) so exp of
                        # one overlaps scores of the other -> ACT stays dense
                        s_e = ps_s.tile([P, 1024], F32, tag="s_e",
                                        name=f"se_{hf}_{pr}_{nk}")
                        s_o = ps_s.tile([P, 1024], F32, tag="s_o",
                                        name=f"so_{hf}_{pr}_{nk}")
                        nks = slice(nk * 128, (nk + 1) * 128)
                        for q2 in range(2):
                            nqs = hf * 1024 + q2 * 512
                            # head-even: PE rows 0:63; head-odd: rows 64:127
                            nc.tensor.matmul(
                                s_e[:, q2 * 512:(q2 + 1) * 512],
                                kT2_sb[0:64, pr, nks],
                                qT_sb[0:64, pr, nqs:nqs + 512],
                                start=True, stop=True,
                            )
                            nc.tensor.matmul(
                                s_o[:, q2 * 512:(q2 + 1) * 512],
                                kT2_sb[64:128, pr, nks],
                                qT_sb[64:128, pr, nqs:nqs + 512],
                                start=True, stop=True,
                            )
                        e_e = expp.tile([P, 1024], BF, tag="e_e")
                        e_o = expp.tile([P, 1024], BF, tag="e_o")
                        nc.scalar.activation(e_e, s_e, Exp, scale=SCALE)
                        nc.scalar.activation(e_o, s_o, Exp, scale=SCALE)
                        for q2 in range(2):
                            # attnV col-tiled pair: even head -> partitions
                            # 0:63, odd head -> 64:127 of the same bank
                            nc.tensor.matmul(
                                o_ps[q2][0:64, :],
                                vaug_sb[:, nk, 2 * pr, :],
                                e_e[:, q2 * 512:(q2 + 1) * 512],
                                start=(nk == 0), stop=(nk == NKT - 1),
                                skip_group_check=True,
                            )
                            nc.tensor.matmul(
                                o_ps[q2][64:128, :],
                                vaug_sb[:, nk, 2 * pr + 1, :],
                                e_o[:, q2 * 512:(q2 + 1) * 512],
                                start=(nk == 0), stop=(nk == NKT - 1),
                                skip_group_check=True,
                            )
                        # denominators: 4 concurrent M=1 col-tiled matmuls
                        for q2 in range(2):
                            for h2 in range(2):
                                r = 32 * (2 * q2 + h2)
                                e_src = e_e if h2 == 0 else e_o
                                nc.tensor.matmul(
                                    d_ps[r:r + 1, :],
                                    ones_sb,
                                    e_src[:, q2 * 512:(q2 + 1) * 512],
                                    start=(nk == 0), stop=(nk == NKT - 1),
                                    tile_position=(0, r),
                                    skip_group_check=True,
                                )
                        if fillers:
                            proj_group(*fillers.pop(), ps_w)

                    # ---- round end: free o banks fast, then normalize ----
                    for q2 in range(2):
                        hq = hf * 1024 + q2 * 512
                        nc.vector.tensor_copy(
                            oTu_sb[:, pr, hq:hq + 512], o_ps[q2])
                    # reciprocal_approx_fast cannot read PSUM on HW: copy
                    # each denom row to SBUF first. partition_broadcast into a
                    # base-64 half is broken on HW: broadcast odd-head recip
                    # to a full 128-partition tile and slice.
                    rpk = [dpkp.tile([1, 512], F32, tag=f"rpk{i}",
                                     name=f"rpk_{hf}_{pr}_{i}")
                           for i in range(4)]
                    for i in range(4):
                        dcp = dpkp.tile([1, 512], F32, tag="dcp",
                                        name=f"dcp_{hf}_{pr}_{i}")
                        nc.vector.tensor_copy(
                            dcp, d_ps[32 * i:32 * i + 1, :])
                        nc.vector.reciprocal_approx_fast(
                            out=rpk[i], in_=dcp)
                    if DEBUG_DUMP:
                        for i in range(4):
                            dma(out=dbg_rpk[4 * hf + pr, i, :], in_=rpk[i])
                    for q2 in range(2):
                        rbe = rbp.tile([64, 512], F32, tag=f"rbe{q2}",
                                       name=f"rbe_{hf}_{pr}_{q2}")
                        rbo = rbp.tile([P, 512], F32, tag=f"rbo{q2}",
                                       name=f"rbo_{hf}_{pr}_{q2}")
                        nc.gpsimd.partition_broadcast(rbe, rpk[2 * q2])
                        nc.gpsimd.partition_broadcast(rbo, rpk[2 * q2 + 1])
                        hq = hf * 1024 + q2 * 512
                        nc.vector.tensor_mul(
                            oT_sb[0:64, pr, hq:hq + 512],
                            oTu_sb[0:64, pr, hq:hq + 512], rbe)
                        nc.vector.tensor_mul(
                            oT_sb[64:128, pr, hq:hq + 512],
                            oTu_sb[64:128, pr, hq:hq + 512],
                            rbo[64:128, :])
                    while fillers:
                        proj_group(*fillers.pop(), ps_w)

        if DEBUG_DUMP:
            dma(out=dbg_qT[:, :, :], in_=qT_sb)
            dma(out=dbg_conv[:, :, :], in_=convT_sb)
            dma(out=dbg_kT2[:, :, :], in_=kT2_sb)
            dma(out=dbg_v[:, :, :, :], in_=vaug_sb)
            dma(out=dbg_oTu[:, :, :], in_=oTu_sb)
            dma(out=dbg_oT[:, :, :], in_=oT_sb)

        # ---- tail: proj for hf=1 queries with freed PSUM banks ----
        with ExitStack() as ps_ctx:
            ps_t = ps_ctx.enter_context(
                tc.tile_pool(name="ps_t", bufs=3, space="PSUM"))
            for nqb in (2, 3):
                for co in range(CT):
                    proj_group(co, nqb, ps_t)

    nc.compile()
    return nc


def kernel(x, q_w, kv_w, sr_w, sr_b, proj_w, proj_b, H=64, W=64, **_kw):
    x = np.asarray(x, dtype=np.float32)
    q_w = np.asarray(q_w, dtype=np.float32)
    kv_w = np.asarray(kv_w, dtype=np.float32)
    sr_w = np.asarray(sr_w, dtype=np.float32)
    sr_b = np.asarray(sr_b, dtype=np.float32)
    proj_w = np.asarray(proj_w, dtype=np.float32)
    proj_b = np.asarray(proj_b, dtype=np.float32)
    B, N, C = x.shape

    if "nc" not in _CACHE:
        _CACHE["nc"] = _build_program()
    nc = _CACHE["nc"]

    bf = ml_dtypes.bfloat16
    qw_t = np.ascontiguousarray(q_w.T).astype(bf)              # [c, dq]
    kw_t = np.ascontiguousarray(kv_w[:512].T).astype(bf)       # [c, dk]
    vw_t = np.ascontiguousarray(kv_w[512:].T).astype(bf)       # [c, dv]
    srw_t = np.ascontiguousarray(
        sr_w.transpose(2, 3, 1, 0).reshape(4, 512, 512)).astype(bf)
    pw_t = np.ascontiguousarray(proj_w.T).astype(bf)           # [c, co]

    in_maps = []
    xT = np.ascontiguousarray(x.transpose(0, 2, 1)).astype(bf)  # [B, C, N]
    for c in range(8):
        b, hf = c // 2, c % 2
        in_maps.append({
            "xq": np.ascontiguousarray(xT[b][:, hf * NQ:(hf + 1) * NQ]),
            "xf": xT[b],
            "qw": qw_t, "kw": kw_t, "vw": vw_t,
            "srw": srw_t, "srb": sr_b,
            "pw": pw_t, "pb": proj_b,
        })

    res = run_bass_kernel_spmd(nc, in_maps, core_ids=list(range(8)))
    _CACHE["last_exec_time_ns"] = res.exec_time_ns

    out = np.empty((B, N, C), dtype=np.float32)
    for c in range(8):
        b, hf = c // 2, c % 2
        out[b, hf * NQ:(hf + 1) * NQ, :] = res.results[c]["out_t"].T
    return out


# revision 6
# speedup vs baseline: 1.4675x; 1.1042x over previous
"""Spatial-reduction attention (PVT-style) on 8 Trainium2 NeuronCores — v2.

Shapes (hardcoded): x [4, 4096, 512], 8 heads, head_dim 64, SR=2 conv
reduction -> 1024 keys. Sharding: core c handles batch c//2, query half
c%2 (2048 queries). Conv + kv recomputed per core pair.

v2 changes over baseline:
  - scores row-tiled: head-even at PE rows 0:63, head-odd at rows 64:127
    (tile_position (0,0)/(64,0)) run CONCURRENTLY -> 2x scores throughput.
    kT packed as [128, kt, keys] (even head dims in partitions 0:63, odd in
    64:127) with no zero padding.
  - one exp per (pr,nk) over a single [128, 2048] PSUM tile (4 banks).
  - attnV col-tiled: v-even (M=64) -> PSUM partitions 0:63, v-odd ->
    64:127 of the SAME bank, concurrent. No ones-column.
  - softmax denominators via 4 concurrent M=1 col-tiled ones-matmuls into
    a separate PSUM bank (rows 0/32/64/96), accumulated over nk.
  - normalize: copy o-banks to SBUF bf16 immediately (frees banks), DMA-
    gather denom rows -> [4,512] reciprocal on DVE -> partition_broadcast
    -> two full-128-lane multiplies in place.
  - PSUM budget: s01 4 banks + o 2 + d 1 + work 1 = 8. proj/late-qT groups
    use the work bank, interleaved into the exp shadow.
"""

import numpy as np
import ml_dtypes
from contextlib import ExitStack

import concourse.bass as bass
import concourse.mybir as mybir
from concourse import bacc
from concourse.bass_utils import run_bass_kernel_spmd
from concourse.tile import TileContext

BF = mybir.dt.bfloat16
F32 = mybir.dt.float32
P = 128
CT = 4            # channel tiles (512 / 128)
NQ = 2048         # queries per core
NKT = 8           # key tiles (1024 / 128)
SCALE = 0.125     # 64 ** -0.5

_CACHE = {}
DEBUG_DUMP = False


def _build_program():
    nc = bacc.Bacc("TRN2", target_bir_lowering=False, debug=False, num_devices=8)

    xq_d = nc.dram_tensor("xq", [512, NQ], BF, kind="ExternalInput")
    xf_d = nc.dram_tensor("xf", [512, 4096], BF, kind="ExternalInput")
    qw_d = nc.dram_tensor("qw", [512, 512], BF, kind="ExternalInput")      # [c, dq]
    kw_d = nc.dram_tensor("kw", [512, 512], BF, kind="ExternalInput")      # [c, dk]
    vw_d = nc.dram_tensor("vw", [512, 512], BF, kind="ExternalInput")      # [c, dv]
    srw_d = nc.dram_tensor("srw", [4, 512, 512], BF, kind="ExternalInput")  # [ij, ci, co]
    srb_d = nc.dram_tensor("srb", [512], F32, kind="ExternalInput")
    pw_d = nc.dram_tensor("pw", [512, 512], BF, kind="ExternalInput")      # [c, co]
    pb_d = nc.dram_tensor("pb", [512], F32, kind="ExternalInput")
    out_d = nc.dram_tensor("out_t", [512, NQ], F32, kind="ExternalOutput")
    if DEBUG_DUMP:
        dbg_qT = nc.dram_tensor("dbg_qT", [P, CT, NQ], BF, kind="ExternalOutput")
        dbg_conv = nc.dram_tensor("dbg_conv", [P, CT, 1024], BF, kind="ExternalOutput")
        dbg_kT2 = nc.dram_tensor("dbg_kT2", [P, CT, 1024], BF, kind="ExternalOutput")
        dbg_v = nc.dram_tensor("dbg_v", [P, NKT, 8, 64], BF, kind="ExternalOutput")
        dbg_oTu = nc.dram_tensor("dbg_oTu", [P, CT, NQ], BF, kind="ExternalOutput")
        dbg_oT = nc.dram_tensor("dbg_oT", [P, CT, NQ], BF, kind="ExternalOutput")
        dbg_rpk = nc.dram_tensor("dbg_rpk", [8, 4, 512], F32, kind="ExternalOutput")
        dbg_e = nc.dram_tensor("dbg_e", [P, 2048], BF, kind="ExternalOutput")

    Exp = mybir.ActivationFunctionType.Exp

    with TileContext(nc) as tc, ExitStack() as ctx:
        const = ctx.enter_context(tc.tile_pool(name="const", bufs=1))
        expp = ctx.enter_context(tc.tile_pool(name="expp", bufs=3))
        dpkp = ctx.enter_context(tc.tile_pool(name="dpkp", bufs=1))
        rbp = ctx.enter_context(tc.tile_pool(name="rbp", bufs=1))
        outp = ctx.enter_context(tc.tile_pool(name="outp", bufs=3))

        dma = nc.sync.dma_start

        # ---- ACT table prewarm (loads exp spline tables during DMA loads) ----
        warm = const.tile([1, 32], F32)
        nc.gpsimd.memset(warm, 0.0)
        warm_o = const.tile([1, 32], BF)
        nc.scalar.activation(warm_o, warm, Exp, scale=1.0)

        # ---- load inputs (conv deps first: srw, then xf chunks) ----
        srw_sb = const.tile([P, 4, CT, 512], BF)
        srw_r = srw_d.rearrange("i (t p) o -> p i t o", p=P)
        for ij4 in range(4):
            dma(out=srw_sb[:, ij4, :, :], in_=srw_r[:, ij4, :, :])
        srb_sb = const.tile([P, CT], F32)
        dma(out=srb_sb, in_=srb_d.rearrange("(t p) -> p t", p=P))
        xf_sb = const.tile([P, CT, 4096], BF)
        xf_r = xf_d.rearrange("(t p) n -> p t n", p=P)
        for t in range(CT):
            dma(out=xf_sb[:, t, :], in_=xf_r[:, t, :])
        qw_sb = const.tile([P, CT, 512], BF)
        qw_r = qw_d.rearrange("(t p) n -> p t n", p=P)
        for t in range(CT):
            dma(out=qw_sb[:, t, :], in_=qw_r[:, t, :])
        xq_sb = const.tile([P, CT, NQ], BF)
        xq_r = xq_d.rearrange("(t p) n -> p t n", p=P)
        for t in range(CT):
            dma(out=xq_sb[:, t, :], in_=xq_r[:, t, :])
        kw_sb = const.tile([P, CT, 512], BF)
        dma(out=kw_sb, in_=kw_d.rearrange("(t p) n -> p t n", p=P))
        vw_sb = const.tile([P, CT, 512], BF)
        dma(out=vw_sb, in_=vw_d.rearrange("(t p) n -> p t n", p=P))
        pw_sb = const.tile([P, CT, 512], BF)
        dma(out=pw_sb, in_=pw_d.rearrange("(t p) n -> p t n", p=P))
        pb_sb = const.tile([P, CT], F32)
        dma(out=pb_sb, in_=pb_d.rearrange("(t p) -> p t", p=P))

        qT_sb = const.tile([P, CT, NQ], BF)
        convT_sb = const.tile([P, CT, 1024], BF)
        kT2_sb = const.tile([P, CT, 1024], BF)
        vaug_sb = const.tile([P, NKT, 8, 64], BF)
        oT_sb = const.tile([P, CT, NQ], BF)
        oTu_sb = const.tile([P, CT, NQ], BF)
        ones_sb = const.tile([P, 1], BF)
        nc.gpsimd.memset(ones_sb, 1.0)

        def qT_group(dq, nqb, pool, tag="work"):
            ps = pool.tile([P, 512], F32, tag=tag,
                           name=f"qt_{dq}_{nqb}")
            for c in range(CT):
                nc.tensor.matmul(
                    ps,
                    qw_sb[:, c, dq * 128:(dq + 1) * 128],
                    xq_sb[:, c, nqb * 512:(nqb + 1) * 512],
                    start=(c == 0), stop=(c == CT - 1),
                )
            nc.vector.tensor_copy(
                qT_sb[:, dq, nqb * 512:(nqb + 1) * 512], ps)

        def proj_group(co, nqb, pool, tag="work"):
            ps = pool.tile([P, 512], F32, tag=tag,
                           name=f"pj_{co}_{nqb}")
            for c in range(CT):
                nc.tensor.matmul(
                    ps,
                    pw_sb[:, c, co * 128:(co + 1) * 128],
                    oT_sb[:, c, nqb * 512:(nqb + 1) * 512],
                    start=(c == 0), stop=(c == CT - 1),
                )
            pt = outp.tile([P, 512], F32)
            nc.vector.tensor_scalar_add(pt, ps, pb_sb[:, co:co + 1])
            dma(out=out_d[co * 128:(co + 1) * 128,
                          nqb * 512:(nqb + 1) * 512], in_=pt)

        def conv_group(co, nkb, pool, tag="ps"):
            ps = pool.tile([P, 512], F32, tag=tag, name=f"cv_{co}_{nkb}")
            n_mm = 0
            # ci-outer so the first matmuls only need the first xf chunk
            for ci in range(CT):
                for ij in range(4):
                    i, j = ij >> 1, ij & 1
                    rhs = xf_sb[:, ci, :].rearrange(
                        "p (a i b j) -> p i j a b", a=32, i=2, b=32, j=2
                    )[:, i, j, nkb * 16:(nkb + 1) * 16, :]
                    nc.tensor.matmul(
                        ps,
                        srw_sb[:, ij, ci, co * 128:(co + 1) * 128],
                        rhs,
                        start=(n_mm == 0), stop=(n_mm == 15),
                    )
                    n_mm += 1
            nc.vector.tensor_scalar_add(
                convT_sb[:, co, nkb * 512:(nkb + 1) * 512],
                ps, srb_sb[:, co:co + 1])

        def kT_group(kt, nkb, pool, tag="ps"):
            ps = pool.tile([P, 512], F32, tag=tag, name=f"kt_{kt}_{nkb}")
            for c in range(CT):
                nc.tensor.matmul(
                    ps,
                    kw_sb[:, c, kt * 128:(kt + 1) * 128],
                    convT_sb[:, c, nkb * 512:(nkb + 1) * 512],
                    start=(c == 0), stop=(c == CT - 1),
                )
            nc.vector.tensor_copy(
                kT2_sb[:, kt, nkb * 512:(nkb + 1) * 512], ps)

        def v_group(nk, pool, tag="ps"):
            ps = pool.tile([P, 512], F32, tag=tag, name=f"v_{nk}")
            for c in range(CT):
                nc.tensor.matmul(
                    ps,
                    convT_sb[:, c, nk * 128:(nk + 1) * 128],
                    vw_sb[:, c, :],
                    start=(c == 0), stop=(c == CT - 1),
                )
            nc.vector.tensor_copy(
                vaug_sb[:, nk, :, :],
                ps.rearrange("p (h e) -> p h e", e=64),
            )

        with ExitStack() as ps_ctx:
            ps1 = ps_ctx.enter_context(tc.tile_pool(name="ps1", bufs=6, space="PSUM"))

            # head: only the first key half (nkb=0) + qT(pr=0); the second
            # half streams into round (0,0)'s PE slack.
            for co in range(CT):
                conv_group(co, 0, ps1)
            for kt in range(CT):
                kT_group(kt, 0, ps1)
            for nk in range(4):
                v_group(nk, ps1)
            for nqb in range(4):
                qT_group(0, nqb, ps1, tag="ps")

        # ---- phase F: attention rounds ----
        with ExitStack() as ps_ctx:
            ps_s = ps_ctx.enter_context(
                tc.tile_pool(name="ps_s", bufs=1, space="PSUM"))
            ps_o = ps_ctx.enter_context(
                tc.tile_pool(name="ps_o", bufs=1, space="PSUM"))
            ps_d = ps_ctx.enter_context(
                tc.tile_pool(name="ps_d", bufs=1, space="PSUM"))
            ps_w = ps_ctx.enter_context(
                tc.tile_pool(name="ps_w", bufs=1, space="PSUM"))

            for hf in range(2):
                for pr in range(4):
                    # late qT: emit right before the round that needs it; runs
                    # in the exp shadow of the previous round.
                    if hf == 0 and pr >= 1:
                        for nqb in range(4):
                            qT_group(pr, nqb, ps_w)
                    # interleave proj(hf=0) groups into hf=1 rounds pr=0,1
                    # proj for hf=0 queries (nqb 0,1) interleaved into the
                    # first two hf=1 rounds; they read oT columns written by
                    # the (complete) hf=0 rounds.
                    fillers = []
                    if hf == 1 and pr < 2:
                        fillers = [(co, pr) for co in range(CT)]

                    o_ps = [ps_o.tile([P, 512], F32, tag=f"o{q2}",
                                      name=f"o_{hf}_{pr}_{q2}")
                            for q2 in range(2)]
                    d_ps = ps_d.tile([P, 512], F32, tag="d",
                                     name=f"d_{hf}_{pr}")
                    for nk in range(NKT):
                        # two 2-bank s tiles split by QUERY half: s[q2] holds
                        # [even-head | odd-head] scores for query chunk q2, so
                        # both MMs of each row/col-tiled pair become ready at
                        # the same instant (concurrent on the PE array) while
                        # exp of the other chunk keeps ACT dense.
                        nks = slice(nk * 128, (nk + 1) * 128)
                        e_q = []
                        for q2 in range(2):
                            s_q = ps_s.tile([P, 1024], F32, tag=f"s{q2}",
                                            name=f"s_{hf}_{pr}_{nk}_{q2}")
                            nqs = hf * 1024 + q2 * 512
                            # head-even: PE rows 0:63; head-odd: rows 64:127
                            nc.tensor.matmul(
                                s_q[:, 0:512],
                                kT2_sb[0:64, pr, nks],
                                qT_sb[0:64, pr, nqs:nqs + 512],
                                start=True, stop=True,
                            )
                            nc.tensor.matmul(
                                s_q[:, 512:1024],
                                kT2_sb[64:128, pr, nks],
                                qT_sb[64:128, pr, nqs:nqs + 512],
                                start=True, stop=True,
                            )
                            e = expp.tile([P, 1024], BF, tag=f"e{q2}")
                            nc.scalar.activation(e, s_q, Exp, scale=SCALE)
                            e_q.append(e)
                        for q2 in range(2):
                            # attnV col-tiled pair: even head -> partitions
                            # 0:63, odd head -> 64:127 of the same bank
                            nc.tensor.matmul(
                                o_ps[q2][0:64, :],
                                vaug_sb[:, nk, 2 * pr, :],
                                e_q[q2][:, 0:512],
                                start=(nk == 0), stop=(nk == NKT - 1),
                                skip_group_check=True,
                            )
                            nc.tensor.matmul(
                                o_ps[q2][64:128, :],
                                vaug_sb[:, nk, 2 * pr + 1, :],
                                e_q[q2][:, 512:1024],
                                start=(nk == 0), stop=(nk == NKT - 1),
                                skip_group_check=True,
                            )
                            # denominator pair for this query chunk
                            for h2 in range(2):
                                r = 32 * (2 * q2 + h2)
                                nc.tensor.matmul(
                                    d_ps[r:r + 1, :],
                                    ones_sb,
                                    e_q[q2][:, h2 * 512:(h2 + 1) * 512],
                                    start=(nk == 0), stop=(nk == NKT - 1),
                                    tile_position=(0, r),
                                    skip_group_check=True,
                                )
                        if fillers:
                            proj_group(*fillers.pop(), ps_w)
                        if hf == 0 and pr == 0 and nk == 1:
                            # second key half: conv/kT/v stream into the PE
                            # slack of this round (needed from nk=4 on)
                            for co in range(CT):
                                conv_group(co, 1, ps_w, tag="work")
                            for kt in range(CT):
                                kT_group(kt, 1, ps_w, tag="work")
                            for nk2 in range(4, 8):
                                v_group(nk2, ps_w, tag="work")

                    # ---- round end: free o banks fast, then normalize ----
                    for q2 in range(2):
                        hq = hf * 1024 + q2 * 512
                        nc.vector.tensor_copy(
                            oTu_sb[:, pr, hq:hq + 512], o_ps[q2])
                    # reciprocal_approx_fast cannot read PSUM on HW: copy
                    # each denom row to SBUF first. partition_broadcast into a
                    # base-64 half is broken on HW: broadcast odd-head recip
                    # to a full 128-partition tile and slice.
                    rpk = [dpkp.tile([1, 512], F32, tag=f"rpk{i}",
                                     name=f"rpk_{hf}_{pr}_{i}")
                           for i in range(4)]
                    for i in range(4):
                        dcp = dpkp.tile([1, 512], F32, tag="dcp",
                                        name=f"dcp_{hf}_{pr}_{i}")
                        nc.vector.tensor_copy(
                            dcp, d_ps[32 * i:32 * i + 1, :])
                        nc.vector.reciprocal_approx_fast(
                            out=rpk[i], in_=dcp)
                    if DEBUG_DUMP:
                        for i in range(4):
                            dma(out=dbg_rpk[4 * hf + pr, i, :], in_=rpk[i])
                    for q2 in range(2):
                        rbe = rbp.tile([64, 512], F32, tag=f"rbe{q2}",
                                       name=f"rbe_{hf}_{pr}_{q2}")
                        rbo = rbp.tile([P, 512], F32, tag=f"rbo{q2}",
                                       name=f"rbo_{hf}_{pr}_{q2}")
                        nc.gpsimd.partition_broadcast(rbe, rpk[2 * q2])
                        nc.gpsimd.partition_broadcast(rbo, rpk[2 * q2 + 1])
                        hq = hf * 1024 + q2 * 512
                        nc.vector.tensor_mul(
                            oT_sb[0:64, pr, hq:hq + 512],
                            oTu_sb[0:64, pr, hq:hq + 512], rbe)
                        nc.vector.tensor_mul(
                            oT_sb[64:128, pr, hq:hq + 512],
                            oTu_sb[64:128, pr, hq:hq + 512],
                            rbo[64:128, :])
                    while fillers:
                        proj_group(*fillers.pop(), ps_w)

        if DEBUG_DUMP:
            dma(out=dbg_qT[:, :, :], in_=qT_sb)
            dma(out=dbg_conv[:, :, :], in_=convT_sb)
            dma(out=dbg_kT2[:, :, :], in_=kT2_sb)
            dma(out=dbg_v[:, :, :, :], in_=vaug_sb)
            dma(out=dbg_oTu[:, :, :], in_=oTu_sb)
            dma(out=dbg_oT[:, :, :], in_=oT_sb)

        # ---- tail: proj for hf=1 queries with freed PSUM banks ----
        with ExitStack() as ps_ctx:
            ps_t = ps_ctx.enter_context(
                tc.tile_pool(name="ps_t", bufs=3, space="PSUM"))
            for nqb in (2, 3):
                for co in range(CT):
                    proj_group(co, nqb, ps_t)

    nc.compile()
    return nc


def kernel(x, q_w, kv_w, sr_w, sr_b, proj_w, proj_b, H=64, W=64, **_kw):
    x = np.asarray(x, dtype=np.float32)
    q_w = np.asarray(q_w, dtype=np.float32)
    kv_w = np.asarray(kv_w, dtype=np.float32)
    sr_w = np.asarray(sr_w, dtype=np.float32)
    sr_b = np.asarray(sr_b, dtype=np.float32)
    proj_w = np.asarray(proj_w, dtype=np.float32)
    proj_b = np.asarray(proj_b, dtype=np.float32)
    B, N, C = x.shape

    if "nc" not in _CACHE:
        _CACHE["nc"] = _build_program()
    nc = _CACHE["nc"]

    bf = ml_dtypes.bfloat16
    qw_t = np.ascontiguousarray(q_w.T).astype(bf)              # [c, dq]
    kw_t = np.ascontiguousarray(kv_w[:512].T).astype(bf)       # [c, dk]
    vw_t = np.ascontiguousarray(kv_w[512:].T).astype(bf)       # [c, dv]
    srw_t = np.ascontiguousarray(
        sr_w.transpose(2, 3, 1, 0).reshape(4, 512, 512)).astype(bf)
    pw_t = np.ascontiguousarray(proj_w.T).astype(bf)           # [c, co]

    in_maps = []
    xT = np.ascontiguousarray(x.transpose(0, 2, 1)).astype(bf)  # [B, C, N]
    for c in range(8):
        b, hf = c // 2, c % 2
        in_maps.append({
            "xq": np.ascontiguousarray(xT[b][:, hf * NQ:(hf + 1) * NQ]),
            "xf": xT[b],
            "qw": qw_t, "kw": kw_t, "vw": vw_t,
            "srw": srw_t, "srb": sr_b,
            "pw": pw_t, "pb": proj_b,
        })

    res = run_bass_kernel_spmd(nc, in_maps, core_ids=list(range(8)))
    _CACHE["last_exec_time_ns"] = res.exec_time_ns

    out = np.empty((B, N, C), dtype=np.float32)
    for c in range(8):
        b, hf = c // 2, c % 2
        out[b, hf * NQ:(hf + 1) * NQ, :] = res.results[c]["out_t"].T
    return out
